# revision 1
# baseline (speedup 1.0000x reference)
"""Grouped Conv2D (G=8, 3x3, SAME) on 8 TRN2 NeuronCores via Bass/Tile.

Sharding: data-parallel over batch (32 images -> 4 per core).

Two complementary layouts, balanced so the serial DMA-engine stream (the
bottleneck) shrinks while the tensor engine's idle slack absorbs the extra
matmul rows:

Scheme A (groups 2..7): host deinterleaves the padded input width by
stride 2 into 4 shifted row-copies (2x input duplication) so SBUF
partitions hold (wp in 0..3, ci in 0..31) for one group; one matmul covers
all 3 kw taps for a 2x2 spatial output block packed on PSUM partitions
(hq, wq, co); the 3 kh taps ride on 4 accumulating h-shifted matmuls.
4 streamed rows per 128 outputs.

Scheme N (groups 0..1): no input duplication - partitions hold
(gp in 0..1, wp in 0..1, ci) for the group PAIR, PSUM packs (gp, wq, co),
and 6 accumulating matmuls (3 kh row-shifts x 2 w-block shifts) cover the
taps. 6 rows per 128 outputs, but ~half the input bytes for the region.

Everything runs in bf16 (fp32 PSUM accumulate). Weights for A are stored
once per kh in overlapping slots (the per-shift stationary matrix is a
2-slot slice); input tiles carry only the 56 real rows with the
pad-touching matmuls shrinking their tau range; dummy matmuls warm the PE
p-state during the DMA lead-in; output DMAs are gated behind the input
stream so they never starve the PE.
"""

import numpy as np
import ml_dtypes

import concourse.bass as bass
import concourse.mybir as mybir
import concourse.tile as tile
from concourse.bass_utils import run_bass_kernel_spmd
from concourse.vector_clock import ScopedClock

# Problem constants (hardcoded per harness contract).
B, H, W, C = 32, 56, 56, 256
G = 8
GA = 6  # groups handled by scheme A (2..7)
CPG = C // G  # 32
KH = KW = 3
NCORES = 8
BC = B // NCORES  # batches per core
T = 28  # stride-2 w blocks per row (scheme A)
TN = 29  # stride-2 w blocks, no dup (scheme N)
NWARM = 8  # PE p-state warmup dummy matmuls

_F32 = mybir.dt.float32
_BF16 = mybir.dt.bfloat16
_BF16NP = np.dtype(ml_dtypes.bfloat16)

# Scheme N matmul order: (kh, dt); first/last are always full-range so they
# can carry the PSUM start/stop flags.
_N_TAPS = ((1, 0), (0, 0), (0, 1), (2, 0), (2, 1), (1, 1))


def _max_waits(inst):
    # This container's walrus rejects instructions carrying several sync
    # waits ("Too many sync wait commands"); matmul lowers through the
    # LDWEIGHTS struct which is strictest, and the SP drain's NO_STRUCT
    # encoding also rejects them, so give those zero embedded waits.
    if isinstance(inst, (mybir.InstMatmult, mybir.InstDrain)):
        return 0
    return 1


def _split_sync_waits(nc):
    """Hoist excess sync waits onto same-engine nops placed just before
    the owning instruction (program order on one sequencer preserves the
    wait semantics)."""
    cnt = 0
    for bb in nc.m.functions[0].blocks:
        insts = list(bb.instructions)
        if not any(
            inst.sync_info is not None
            and len(inst.sync_info.on_wait) > _max_waits(inst)
            for inst in insts
        ):
            continue
        newl = []
        for inst in insts:
            si = inst.sync_info
            waits = list(si.on_wait) if si is not None else []
            maxw = _max_waits(inst)
            if len(waits) > maxw:
                for wv in waits[maxw:]:
                    cnt += 1
                    nop = mybir.InstNoOp(
                        name=f"waitsplit-{cnt}",
                        engine=inst.engine,
                        bass_nofuse=True,
                        sync_info=mybir.SyncInfo(on_wait=[wv], on_update=[]),
                    )
                    nc.register_instruction(nop, overwrite=True)
                    newl.append(nop)
                inst.sync_info = mybir.SyncInfo(
                    on_wait=waits[:maxw], on_update=list(si.on_update)
                )
            newl.append(inst)
        live = bb.instructions
        live.clear()
        for inst in newl:
            bb.add_instruction(inst)


def _patch_tile_drain():
    if getattr(tile.TileContext, "_drain_patch_applied", False):
        return

    def _drain_and_barrier(self, tick_clock, wait_clock):
        nc = self.nc
        probe = nc.sync.nop(nofuse=True)
        wait_clock.add_sem_waits(
            probe.ins, ScopedClock({None: tick_clock.global_clock})
        )
        nc.sync.drain()
        nc.all_engine_barrier()
        assert self.sems is not None
        popped = nc._tile_sem_poison_stack.pop()
        assert popped is self._sem_poison
        nc.clear_and_free_semaphores(list(self.sems.allocated().values()))
        _split_sync_waits(nc)

    tile.TileContext._drain_and_barrier = _drain_and_barrier
    tile.TileContext._drain_patch_applied = True


# Scheme A per-(cc, dh) matmul read geometry: (tau_lo, tau_hi, u0, v).
# Moving rows live at h = 28*cc + 2*tau + dh; the tile stores r = h-1 as
# [u(28), v(2)]. dh=0 at cc=0 and dh=3 at cc=1 would touch the (absent,
# all-zero) h-pad rows at tau=0 / tau=13, so those shrink their tau range.
def _mm_geom(cc, dh):
    if dh == 0:
        tlo = 1 if cc == 0 else 0
        return tlo, 14, 14 * cc + tlo - 1, 1
    if dh == 1:
        return 0, 14, 14 * cc, 0
    if dh == 2:
        return 0, 14, 14 * cc, 1
    thi = 14 if cc == 0 else 13
    return 0, thi, 14 * cc + 1, 0


def build_bass(nwarm=NWARM):
    """One SPMD Bass program; every core runs it on its own batch shard."""
    _patch_tile_drain()
    nc = bass.Bass("TRN2", target_bir_lowering=False, debug=False,
                   num_devices=NCORES)
    # Scheme A (groups 2..7, ga = g-2):
    # x: [ga, (wp*32+ci), b, u(28), v(2), t(28)]  (r = h_pad-1 = 2u+v)
    x = nc.dram_tensor("x", [GA, 128, BC, T, 2, T], _BF16,
                       kind="ExternalInput")
    # w: [(wp*32+ci), slot(4*GA+1), (wq*32+co)]; per group ga a 5-slot
    # window at 4*ga, slot 4*ga+s holding kh=3-s for s in {1,2,3} and
    # shared zeros at the window edges (multiples of 4). The per-shift
    # stationary matrix is the overlapping 2-slot slice
    # wt[:, 4*ga+3-dh : 4*ga+5-dh, :], which coalesces with the adjacent
    # contiguous 64-dim into the single free dim walrus requires.
    w = nc.dram_tensor("w", [128, 4 * GA + 1, 64], _BF16,
                       kind="ExternalInput")
    # y: [ga, (hq*64+wq*32+co), b, cc, tau(14), t(28)]
    y = nc.dram_tensor("y", [GA, 128, BC, 2, 14, T], _BF16,
                       kind="ExternalOutput")
    # Scheme N (groups 0..1):
    # xn: [(gp*64+wp*32+ci), b, h(56), tb(29)] = xpad[h+1, 2tb+wp]
    xn = nc.dram_tensor("xn", [128, BC, H, TN], _BF16, kind="ExternalInput")
    # wn: [(gp,wp,ci), kh*2+dt, (gp,wq,co)]
    wn = nc.dram_tensor("wn", [128, 6, 128], _BF16, kind="ExternalInput")
    # yn: [(gp*64+wq*32+co), b, cci(4), tau(14), t(28)]
    yn = nc.dram_tensor("yn", [128, BC, 4, 14, T], _BF16,
                        kind="ExternalOutput")

    with tile.TileContext(nc) as tc:
        with (
            tc.tile_pool(name="wpool", bufs=1) as wpool,
            tc.tile_pool(name="xpool", bufs=1) as xpool,
            tc.tile_pool(name="ypool", bufs=1) as ypool,
            tc.tile_pool(name="psum", bufs=6, space=bass.MemorySpace.PSUM) as pp,
            tc.tile_pool(name="psumd", bufs=1,
                         space=bass.MemorySpace.PSUM) as ppd,
        ):
            # PE p-state warmup: matmuls on zeroed scratch keep the tensor
            # engine continuously busy from ~0.3us until the first real
            # operands land, so the ramp (mid pstate until 3us of sustained
            # use) is paid on dummies instead of real work.
            dumw = wpool.tile([128, 128], _BF16, tag="dumw")
            dumx = wpool.tile([128, 128], _BF16, tag="dumx")
            nc.gpsimd.memset(dumw[:], 0)
            nc.gpsimd.memset(dumx[:], 0)
            psd = ppd.tile([128, 128], _F32, tag="dum")
            for _ in range(nwarm):
                nc.tensor.matmul(psd[:, :], dumw[:, :], dumx[:, :],
                                 start=True, stop=True)

            wtn = wpool.tile([128, 6, 128], _BF16, tag="wtn")
            wt = wpool.tile([128, 4 * GA + 1, 64], _BF16)

            xnts = {}
            for b in range(BC):
                xnts[b] = xpool.tile([128, H, TN], _BF16, tag=f"xn{b}",
                                     name=f"xnt_{b}")
            xts = {}
            for ga in range(GA):
                for b in range(BC):
                    xts[(ga, b)] = xpool.tile([128, T, 2, T], _BF16,
                                              tag=f"x{ga}_{b}",
                                              name=f"xt_{ga}_{b}")

            # Input DMA stream: scheme-N tiles + weights lead (the PE runs
            # the N region first), then the A tiles with the A weights
            # early. Output DMAs are gated behind the tail of the input
            # stream (chain_iter_dep) so their transfer requests queue
            # after every input's - FIFO on the serial DMA engines then
            # keeps the PE fed - yet start right as the inputs drain.
            # First tile in two parts so the PE's first psum block (rows
            # 0..15) can start before the whole tile lands.
            nc.sync.dma_start(xnts[0][:, 0:29, :], xn[:, 0, 0:29])
            nc.sync.dma_start(wtn[:], wn[:])
            # Short transfers early in the stream drain the SP issue pipe
            # (one DMA issued ~every 0.59us); keep the longer xn1 ahead of
            # the small xn0 remainder so the DMA engines stay busy.
            nc.sync.dma_start(xnts[1][:], xn[:, 1])
            nc.sync.dma_start(xnts[0][:, 29:H, :], xn[:, 0, 29:H])
            for b in range(2, BC):
                nc.sync.dma_start(xnts[b][:], xn[:, b])
            gate_in = None
            for i, (ga, b) in enumerate([(ga, b) for ga in range(GA)
                                         for b in range(BC)]):
                d = nc.sync.dma_start(xts[(ga, b)][:], x[ga, :, b])
                if i == 1:
                    nc.sync.dma_start(wt[:], w[:])
                if (ga, b) == (GA - 1, 1):
                    gate_in = d
            tc.chain_iter_dep("iogate", gate_in.ins)

            ci = 0
            first_out = True

            # --- Scheme N region: groups 0..1, all images ---
            for b in range(BC):
                ytn = ypool.tile([128, 4, 14, T], _BF16, tag=f"yn{b}")
                xnt = xnts[b]
                for cci in range(4):
                    ps = pp.tile([128, 14, T], _F32, tag="ps")
                    for kh, dt in _N_TAPS:
                        tlo = 1 if (kh == 0 and cci == 0) else 0
                        thi = 13 if (kh == 2 and cci == 3) else 14
                        r0 = 14 * cci + tlo + kh - 1
                        n = thi - tlo
                        nc.tensor.matmul(
                            ps[:, tlo:thi, :],
                            wtn[:, kh * 2 + dt, :],
                            xnt[:, r0:r0 + n, dt:dt + T],
                            start=(kh == 1 and dt == 0),
                            stop=(kh == 1 and dt == 1),
                        )
                    dst = ytn[:, cci]
                    if ci % 2 == 0:
                        nc.vector.tensor_copy(dst, ps[:, :, :])
                    else:
                        nc.scalar.copy(dst, ps[:, :, :])
                    ci += 1
                od = nc.sync.dma_start(yn[:, b], ytn[:])
                if first_out:
                    tc.chain_iter_dep("iogate", od.ins)
                    first_out = False

            # --- Scheme A region: groups 2..7 ---
            for ga in range(GA):
                split_out = ga == GA - 1  # finer tail so the last PSUM
                # chunk's store is small and the stream ends sooner
                if not split_out:
                    yt = ypool.tile([128, BC, 2, 14, T], _BF16, tag=f"y{ga}")
                for b in range(BC):
                    split_cc = split_out and b >= BC - 2
                    if split_out and not split_cc:
                        yt = ypool.tile([128, 1, 2, 14, T], _BF16,
                                        tag=f"y{ga}_{b}",
                                        name=f"yt_{ga}_{b}")
                    xt = xts[(ga, b)]
                    for cc in range(2):
                        if split_cc:
                            yt = ypool.tile([128, 1, 1, 14, T], _BF16,
                                            tag=f"y{ga}_{b}_{cc}",
                                            name=f"yt_{ga}_{b}_{cc}")
                        ps = pp.tile([128, 14, T], _F32, tag="ps")
                        # dh order: 1 (start, always full range), the two
                        # possibly-partial shifts, then 2 (stop, full).
                        for dh in (1, 0, 3, 2):
                            tlo, thi, u0, v = _mm_geom(cc, dh)
                            n = thi - tlo
                            nc.tensor.matmul(
                                ps[:, tlo:thi, :],
                                wt[:, 4 * ga + 3 - dh:4 * ga + 5 - dh, :],
                                xt[:, u0:u0 + n, v, :],
                                start=(dh == 1),
                                stop=(dh == 2),
                            )
                        if split_cc:
                            dst = yt[:, 0, 0]
                        else:
                            dst = yt[:, 0 if split_out else b, cc]
                        if ci % 2 == 0:
                            nc.vector.tensor_copy(dst, ps[:, :, :])
                        else:
                            nc.scalar.copy(dst, ps[:, :, :])
                        ci += 1
                        if split_cc:
                            nc.sync.dma_start(
                                y[ga, :, b:b + 1, cc:cc + 1], yt[:])
                    if split_out and not split_cc:
                        nc.sync.dma_start(y[ga, :, b:b + 1], yt[:])
                if not split_out:
                    nc.sync.dma_start(y[ga], yt[:])
    return nc


_NC_CACHE = None


def _get_nc():
    global _NC_CACHE
    if _NC_CACHE is None:
        _NC_CACHE = build_bass()
    return _NC_CACHE


def _pack_x(inputs):
    """A-region: [B,H,W,192(g2..7)] fp32 -> [GA, 128(wp,ci), B, 28, 2, 28]."""
    xpad = np.zeros((B, H, W + 2, GA * CPG), np.float32)
    xpad[:, :, 1:W + 1, :] = inputs[..., 2 * CPG:]
    s = xpad.strides
    # xv[b, h, t, wp, ga, ci] = xpad[b, h, 2t+wp, 32ga+ci]
    xv = np.lib.stride_tricks.as_strided(
        xpad, shape=(B, H, T, 4, GA, CPG),
        strides=(s[0], s[1], 2 * s[2], s[2], CPG * s[3], s[3]))
    xt = xv.transpose(4, 3, 5, 0, 1, 2).reshape(GA, 128, B, T, 2, T)
    return np.ascontiguousarray(xt).astype(_BF16NP)


def _pack_xn(inputs):
    """N-region: [B,H,W,64(g0..1)] fp32 -> [128(gp,wp,ci), B, 56, 29]."""
    xpad = np.zeros((B, H, W + 2, 2 * CPG), np.float32)
    xpad[:, :, 1:W + 1, :] = inputs[..., :2 * CPG]
    s = xpad.strides
    # xv[b, h, tb, wp, gp, ci] = xpad[b, h, 2tb+wp, 32gp+ci]
    xv = np.lib.stride_tricks.as_strided(
        xpad, shape=(B, H, TN, 2, 2, CPG),
        strides=(s[0], s[1], 2 * s[2], s[2], CPG * s[3], s[3]))
    xt = xv.transpose(4, 3, 5, 0, 1, 2).reshape(128, B, H, TN)
    return np.ascontiguousarray(xt).astype(_BF16NP)


def _pack_w(kern):
    """HWIO [3,3,32,256] -> A: [128(wp,ci), slot(4*GA+1), 64(wq,co)] bf16;
    slot 4*ga+s holds kh=3-s for s in {1,2,3}, zeros at multiples of 4."""
    wd = np.zeros((128, 4 * GA + 1, 64), np.float32)
    for ga in range(GA):
        g = ga + 2
        for s in (1, 2, 3):
            kh = 3 - s
            for wq in range(2):
                for wp in range(4):
                    kw = wp - wq
                    if not 0 <= kw < KW:
                        continue
                    wd[wp * 32:(wp + 1) * 32, 4 * ga + s,
                       wq * 32:wq * 32 + 32] = \
                        kern[kh, kw, :, g * CPG:(g + 1) * CPG]
    return wd.astype(_BF16NP)


def _pack_wn(kern):
    """HWIO -> N: [128(gp,wp,ci), kh*2+dt, 128(gp,wq,co)] bf16 (groups 0,1,
    block-diagonal over the pair)."""
    wd = np.zeros((128, 6, 128), np.float32)
    for kh in range(KH):
        for dt in range(2):
            for gp in range(2):
                for wp in range(2):
                    for wq in range(2):
                        kw = 2 * dt + wp - wq
                        if not 0 <= kw < KW:
                            continue
                        wd[gp * 64 + wp * 32:gp * 64 + wp * 32 + 32,
                           kh * 2 + dt,
                           gp * 64 + wq * 32:gp * 64 + wq * 32 + 32] = \
                            kern[kh, kw, :, gp * CPG:(gp + 1) * CPG]
    return wd.astype(_BF16NP)


def _make_in_maps(inputs, kern):
    inputs = np.asarray(inputs, np.float32)
    kern = np.asarray(kern, np.float32)
    xp = _pack_x(inputs)
    xpn = _pack_xn(inputs)
    wd = _pack_w(kern)
    wdn = _pack_wn(kern)
    return [
        {
            "x": np.ascontiguousarray(xp[:, :, c * BC:(c + 1) * BC]),
            "xn": np.ascontiguousarray(xpn[:, c * BC:(c + 1) * BC]),
            "w": wd,
            "wn": wdn,
        }
        for c in range(NCORES)
    ]


def _unpack_y(ya, ync):
    """A [GA,128,BC,2,14,28] + N [128,BC,4,14,28] -> [BC,H,W,C] fp32."""
    oa = np.asarray(ya, np.float32).reshape(GA, 2, 2, CPG, BC, 2, 14, T)
    outa = oa.transpose(4, 5, 6, 1, 7, 2, 0, 3).reshape(BC, H, W, GA * CPG)
    on = np.asarray(ync, np.float32).reshape(2, 2, CPG, BC, 4, 14, T)
    outn = on.transpose(3, 4, 5, 6, 1, 0, 2).reshape(BC, H, W, 2 * CPG)
    return np.concatenate([outn, outa], axis=3)


def kernel(inputs, kernel, bias):
    nc = _get_nc()
    in_maps = _make_in_maps(inputs, kernel)
    try:
        res = run_bass_kernel_spmd(nc, in_maps, list(range(NCORES)))
    except ModuleNotFoundError:
        # BASS_TRACE set but the axon NTFF hook module is absent in this
        # container; retry with tracing suppressed.
        import os

        os.environ["BASS_NEVER_TRACE"] = "1"
        res = run_bass_kernel_spmd(nc, in_maps, list(range(NCORES)))

    outs = [
        _unpack_y(res.results[c]["y"], res.results[c]["yn"])
        for c in range(NCORES)
    ]
    out = np.concatenate(outs, axis=0)
    out = out + np.asarray(bias, np.float32)
    return out.astype(np.float32)



# revision 13
# speedup vs baseline: 1.0965x; 1.0965x over previous
"""Grouped Conv2D (G=8, 3x3, SAME) on 8 TRN2 NeuronCores via Bass/Tile.

Sharding: data-parallel over batch (32 images -> 4 per core).

Quadrant (space-to-depth) scheme, uniform for all 8 groups: SBUF
partitions hold (hp, wp, ci) -- the 4 pixels of a stride-2 2x2 input
block for one group -- with zero input duplication. PSUM partitions
pack (hq, wq, co): the 2x2 output block. Four accumulating matmuls
per PSUM block, one per moving-tile shift (dh, dw) in {0,1}^2, cover
all 9 taps exactly once: kh = 2*dh + hp - hq, kw = 2*dw + wp - wq.
This is the minimum: an input pixel is a tap of 4 distinct output
blocks, and with no duplication each shift serves exactly one of
them, so 4 shifted matmuls per block is a floor. One PE row per
2x2-output-pixel-column: 8g*4b*2cc*4 = 256 matmuls of 392 rows =
100352 rows.

Everything runs in bf16 (fp32 PSUM accumulate). Input tiles are
host-packed padded (29x29 per quadrant) so every DMA moves >=512B
descriptors. Per-group weight DMAs ride just ahead of their input
tile; dummy matmuls on an unread PSUM bank warm the PE p-state
during the DMA lead-in; output DMAs sit after all input DMAs in SP
program order so their transfer requests queue behind every input's
on the serial DMA engines; the final (g,b) outputs split per-cc so
the last PSUM chunk's store is small and the stream ends sooner.
"""

import numpy as np
import ml_dtypes

import concourse.bass as bass
import concourse.mybir as mybir
import concourse.tile as tile
from concourse.bass_utils import run_bass_kernel_spmd
from concourse.vector_clock import ScopedClock

# Problem constants (hardcoded per harness contract).
B, H, W, C = 32, 56, 56, 256
G = 8
CPG = C // G  # 32
KH = KW = 3
NCORES = 8
BC = B // NCORES  # 4 batches per core
HB = 29  # padded stride-2 tile extent (rows -1..56 -> 29 pairs)

_F32 = mybir.dt.float32
_BF16 = mybir.dt.bfloat16
_BF16NP = np.dtype(ml_dtypes.bfloat16)


def _max_waits(inst):
    # This container's walrus rejects instructions carrying several sync
    # waits ("Too many sync wait commands"); matmul lowers through the
    # LDWEIGHTS struct which is strictest, and the SP drain's NO_STRUCT
    # encoding also rejects them, so give those zero embedded waits.
    if isinstance(inst, (mybir.InstMatmult, mybir.InstDrain)):
        return 0
    return 1


def _split_sync_waits(nc):
    """Hoist excess sync waits onto same-engine nops placed just before
    the owning instruction (program order on one sequencer preserves the
    wait semantics)."""
    cnt = 0
    for bb in nc.m.functions[0].blocks:
        insts = list(bb.instructions)
        if not any(
            inst.sync_info is not None
            and len(inst.sync_info.on_wait) > _max_waits(inst)
            for inst in insts
        ):
            continue
        newl = []
        for inst in insts:
            si = inst.sync_info
            waits = list(si.on_wait) if si is not None else []
            maxw = _max_waits(inst)
            if len(waits) > maxw:
                for wv in waits[maxw:]:
                    cnt += 1
                    nop = mybir.InstNoOp(
                        name=f"waitsplit-{cnt}",
                        engine=inst.engine,
                        bass_nofuse=True,
                        sync_info=mybir.SyncInfo(on_wait=[wv], on_update=[]),
                    )
                    nc.register_instruction(nop, overwrite=True)
                    newl.append(nop)
                inst.sync_info = mybir.SyncInfo(
                    on_wait=waits[:maxw], on_update=list(si.on_update)
                )
            newl.append(inst)
        live = bb.instructions
        live.clear()
        for inst in newl:
            bb.add_instruction(inst)


def _patch_tile_drain():
    if getattr(tile.TileContext, "_drain_patch_applied", False):
        return

    def _drain_and_barrier(self, tick_clock, wait_clock):
        nc = self.nc
        probe = nc.sync.nop(nofuse=True)
        wait_clock.add_sem_waits(
            probe.ins, ScopedClock({None: tick_clock.global_clock})
        )
        nc.sync.drain()
        nc.all_engine_barrier()
        assert self.sems is not None
        popped = nc._tile_sem_poison_stack.pop()
        assert popped is self._sem_poison
        nc.clear_and_free_semaphores(list(self.sems.allocated().values()))
        _split_sync_waits(nc)

    tile.TileContext._drain_and_barrier = _drain_and_barrier
    tile.TileContext._drain_patch_applied = True


def build_bass():
    """One SPMD Bass program; every core runs it on its own batch shard."""
    _patch_tile_drain()
    nc = bass.Bass("TRN2", target_bir_lowering=False, debug=False,
                   num_devices=NCORES)
    # x: [g, (hp*64+wp*32+ci), b, hh, ww] with
    #    x[g,(hp,wp,ci),b,hh,ww] = xpad[b, 2hh+hp-1, 2ww+wp-1, 32g+ci]
    x = nc.dram_tensor("x", [G, 128, BC, HB, HB], _BF16,
                       kind="ExternalInput")
    # w: [(hp,wp,ci), g, 2*dh+dw, (hq*64+wq*32+co)] =
    #    kern[2dh+hp-hq, 2dw+wp-wq, ci, 32g+co] where valid, else 0
    w = nc.dram_tensor("w", [128, G, 4, 128], _BF16, kind="ExternalInput")
    # y: [g, (hq*64+wq*32+co), b, cc, h7, t] =
    #    out[b, 2*(14cc+h7)+hq, 2t+wq, 32g+co]
    y = nc.dram_tensor("y", [G, 128, BC, 2, 14, 28], _BF16,
                       kind="ExternalOutput")

    with tile.TileContext(nc) as tc:
        with (
            tc.tile_pool(name="wpool", bufs=1) as wpool,
            tc.tile_pool(name="xpool", bufs=1) as xpool,
            tc.tile_pool(name="ypool", bufs=1) as ypool,
            tc.tile_pool(name="psum", bufs=6, space=bass.MemorySpace.PSUM) as pp,
        ):
            wt = wpool.tile([128, G, 4, 128], _BF16, tag="wt")
            xts = {}
            for g in range(G):
                xts[g] = xpool.tile([128, BC, HB, HB], _BF16, tag=f"x{g}",
                                    name=f"xt_{g}")
            yts = {}
            for g in range(G):
                for b in range(BC):
                    yts[(g, b)] = ypool.tile([128, 2, 14, 28], _BF16,
                                             tag=f"y{g}_{b}",
                                             name=f"yt_{g}_{b}")

            # Input DMA stream, two issue lanes. The head is bound by the
            # serial HWDGE descriptor-generation pipeline (~650ns per DMA),
            # so all weight DMAs ride the gpsimd SWDGE lane, which generates
            # descriptors on the Pool engine in parallel; the SP/HWDGE lane
            # carries only the input tiles. Group 0 is split finely (rows
            # 0:15 cover the whole first PSUM block) so compute starts as
            # early as possible, and per-b so the PE never outruns the
            # issue-limited head of the stream.
            for g in range(G):
                nc.gpsimd.dma_start(wt[:, g], w[:, g])
            nc.sync.dma_start(xts[0][:, 0, 0:15, :], x[0, :, 0, 0:15])
            nc.sync.dma_start(xts[0][:, 0, 15:HB, :], x[0, :, 0, 15:HB])
            nc.sync.dma_start(xts[0][:, 1], x[0, :, 1])
            nc.sync.dma_start(xts[0][:, 2:BC], x[0, :, 2:BC])
            for g in range(1, G):
                nc.sync.dma_start(xts[g][:], x[g])

            # Compute: per (g, b, cc) one PSUM block [128, 14, 28], four
            # shifted accumulating matmuls. Output DMAs are emitted after
            # this loop (SP program order => transfers queue behind all
            # input transfers on the serial DMA engines).
            ci = 0
            for g in range(G):
                for b in range(BC):
                    for cc in range(2):
                        # The very last block is computed as two 196-col
                        # half-blocks so the final PSUM->SBUF copy (on the
                        # tail critical path) is half as long.
                        last = g == G - 1 and b == BC - 1 and cc == 1
                        hsplits = ((0, 7), (7, 14)) if last else ((0, 14),)
                        for h0, h1 in hsplits:
                            ps = pp.tile([128, h1 - h0, 28], _F32, tag="ps")
                            for i, (dh, dw) in enumerate(
                                    ((0, 0), (1, 0), (0, 1), (1, 1))):
                                r0 = 14 * cc + h0 + dh
                                nc.tensor.matmul(
                                    ps[:, :, :],
                                    wt[:, g, 2 * dh + dw, :],
                                    xts[g][:, b, r0:r0 + h1 - h0, dw:dw + 28],
                                    start=(i == 0),
                                    stop=(i == 3),
                                )
                            dst = yts[(g, b)][:, cc, h0:h1]
                            if ci % 2 == 0:
                                nc.vector.tensor_copy(dst, ps[:, :, :])
                            else:
                                nc.scalar.copy(dst, ps[:, :, :])
                            ci += 1

            # Output DMAs, in compute order; final (g,b) pairs split
            # per-cc so the tail transfer after the last copy is short.
            for g in range(G):
                for b in range(BC):
                    if g == G - 1 and b >= BC - 2:
                        nc.sync.dma_start(y[g, :, b, 0:1], yts[(g, b)][:, 0:1])
                        nc.sync.dma_start(y[g, :, b, 1:2], yts[(g, b)][:, 1:2])
                    else:
                        nc.sync.dma_start(y[g, :, b], yts[(g, b)][:])
    return nc


_NC_CACHE = None


def _get_nc():
    global _NC_CACHE
    if _NC_CACHE is None:
        _NC_CACHE = build_bass()
    return _NC_CACHE


def _pack_x(inputs):
    """[B,H,W,C] fp32 -> [G, 128(hp,wp,ci), B, 29, 29] bf16 quadrants."""
    xpad = np.zeros((B, H + 2, W + 2, C), np.float32)
    xpad[:, 1:H + 1, 1:W + 1, :] = inputs
    s = xpad.strides
    # xv[b, hh, hp, ww, wp, g, ci] = xpad[b, 2hh+hp, 2ww+wp, 32g+ci]
    xv = np.lib.stride_tricks.as_strided(
        xpad, shape=(B, HB, 2, HB, 2, G, CPG),
        strides=(s[0], 2 * s[1], s[1], 2 * s[2], s[2], CPG * s[3], s[3]))
    xt = xv.transpose(5, 2, 4, 6, 0, 1, 3).reshape(G, 128, B, HB, HB)
    return np.ascontiguousarray(xt).astype(_BF16NP)


def _pack_w(kern):
    """HWIO [3,3,32,256] -> [128(hp,wp,ci), g, 2dh+dw, 128(hq,wq,co)]."""
    wd = np.zeros((128, G, 4, 128), np.float32)
    for dh in range(2):
        for dw in range(2):
            for hp in range(2):
                for hq in range(2):
                    kh = 2 * dh + hp - hq
                    if not 0 <= kh < KH:
                        continue
                    for wp in range(2):
                        for wq in range(2):
                            kw = 2 * dw + wp - wq
                            if not 0 <= kw < KW:
                                continue
                            for g in range(G):
                                wd[hp * 64 + wp * 32:hp * 64 + wp * 32 + 32,
                                   g, 2 * dh + dw,
                                   hq * 64 + wq * 32:hq * 64 + wq * 32 + 32] \
                                    = kern[kh, kw, :, g * CPG:(g + 1) * CPG]
    return wd.astype(_BF16NP)


def _make_in_maps(inputs, kern):
    inputs = np.asarray(inputs, np.float32)
    kern = np.asarray(kern, np.float32)
    xp = _pack_x(inputs)
    wd = _pack_w(kern)
    return [
        {
            "x": np.ascontiguousarray(xp[:, :, c * BC:(c + 1) * BC]),
            "w": wd,
        }
        for c in range(NCORES)
    ]


def _unpack_y(ya):
    """[G,128,BC,2,14,28] bf16 -> [BC,H,W,C] fp32."""
    o = np.asarray(ya, np.float32).reshape(G, 2, 2, CPG, BC, 2, 14, 28)
    # out[b, 2*(14cc+h7)+hq, 2t+wq, 32g+co]
    return o.transpose(4, 5, 6, 1, 7, 2, 0, 3).reshape(BC, H, W, C)


def kernel(inputs, kernel, bias):
    nc = _get_nc()
    in_maps = _make_in_maps(inputs, kernel)
    try:
        res = run_bass_kernel_spmd(nc, in_maps, list(range(NCORES)))
    except ModuleNotFoundError:
        # BASS_TRACE set but the axon NTFF hook module is absent in this
        # container; retry with tracing suppressed.
        import os

        os.environ["BASS_NEVER_TRACE"] = "1"
        res = run_bass_kernel_spmd(nc, in_maps, list(range(NCORES)))

    outs = [_unpack_y(res.results[c]["y"]) for c in range(NCORES)]
    out = np.concatenate(outs, axis=0)
    out = out + np.asarray(bias, np.float32)
    return out.astype(np.float32)


# revision 19
# speedup vs baseline: 1.2448x; 1.1353x over previous
"""Grouped Conv2D (G=8, 3x3, SAME) on 8 TRN2 NeuronCores via Bass/Tile.

Sharding: data-parallel over batch (32 images -> 4 per core).

Quadrant (space-to-depth) scheme, uniform for all 8 groups: SBUF
partitions hold (hp, wp, ci) -- the 4 pixels of a stride-2 2x2 input
block for one group -- with zero input duplication. PSUM partitions
pack (hq, wq, co): the 2x2 output block. Taps decompose over moving
shifts (dh, dw) in {0,1}^2: kh = 2*dh + hp - hq, kw = 2*dw + wp - wq,
each (tap, output) pair covered exactly once.

Compute runs in fp8 (e4m3) DoubleRow matmuls at 2x PE rate, with
error compensation: x = xh + xl (fp8 value + fp8 residual), W split
likewise (scaled by 16 for subnormal headroom; host divides the
output by 16). y = Wh*xh + Wh*xl + Wl*xh leaves only O(eps^2)
error (~3e-3 relative, same as a plain bf16 kernel). DoubleRow
contracts a leading pair dim on both operands (out = sum_i
W[:,i].T @ X[:,i]); the pair dim carries the two dh shifts as an
aliased-stride view of the input tile, so per PSUM block the whole
sum is 6 half-rate matmuls (3 terms x 2 dw) instead of 8.

The kernel is DMA-bound (~39us of HBM traffic vs ~31us of PE), so
the schedule holds the serial DMA engine stream: weight DMAs ride
the gpsimd SWDGE lane (the SP/HWDGE lane's ~650ns-per-DMA issue
pipeline binds the head), input tiles stream per-group with group 0
split finely, and output DMAs sit after all input DMAs in SP
program order so their transfer requests queue behind every input's.
"""

import numpy as np
import ml_dtypes

import concourse.bass as bass
import concourse.mybir as mybir
import concourse.tile as tile
from concourse.ap import AP
from concourse.bass_utils import run_bass_kernel_spmd
from concourse.vector_clock import ScopedClock

# Problem constants (hardcoded per harness contract).
B, H, W, C = 32, 56, 56, 256
G = 8
CPG = C // G  # 32
KH = KW = 3
NCORES = 8
BC = B // NCORES  # 4 batches per core
HB = 29  # padded stride-2 tile extent (rows -1..56 -> 29 pairs)
WSCALE = 16.0  # weight pre-scale; host divides the output by it

_F32 = mybir.dt.float32
_BF16 = mybir.dt.bfloat16
_FP8 = mybir.dt.float8e4
_BF16NP = np.dtype(ml_dtypes.bfloat16)
_FP8NP = np.dtype(ml_dtypes.float8_e4m3)
_DR = mybir.MatmulPerfMode.DoubleRow


def _max_waits(inst):
    # This container's walrus rejects instructions carrying several sync
    # waits ("Too many sync wait commands"); matmul lowers through the
    # LDWEIGHTS struct which is strictest, and the SP drain's NO_STRUCT
    # encoding also rejects them, so give those zero embedded waits.
    if isinstance(inst, (mybir.InstMatmult, mybir.InstDrain)):
        return 0
    return 1


def _split_sync_waits(nc):
    """Hoist excess sync waits onto same-engine nops placed just before
    the owning instruction (program order on one sequencer preserves the
    wait semantics)."""
    cnt = 0
    for bb in nc.m.functions[0].blocks:
        insts = list(bb.instructions)
        if not any(
            inst.sync_info is not None
            and len(inst.sync_info.on_wait) > _max_waits(inst)
            for inst in insts
        ):
            continue
        newl = []
        for inst in insts:
            si = inst.sync_info
            waits = list(si.on_wait) if si is not None else []
            maxw = _max_waits(inst)
            if len(waits) > maxw:
                for wv in waits[maxw:]:
                    cnt += 1
                    nop = mybir.InstNoOp(
                        name=f"waitsplit-{cnt}",
                        engine=inst.engine,
                        bass_nofuse=True,
                        sync_info=mybir.SyncInfo(on_wait=[wv], on_update=[]),
                    )
                    nc.register_instruction(nop, overwrite=True)
                    newl.append(nop)
                inst.sync_info = mybir.SyncInfo(
                    on_wait=waits[:maxw], on_update=list(si.on_update)
                )
            newl.append(inst)
        live = bb.instructions
        live.clear()
        for inst in newl:
            bb.add_instruction(inst)


def _patch_tile_drain():
    if getattr(tile.TileContext, "_drain_patch_applied", False):
        return

    def _drain_and_barrier(self, tick_clock, wait_clock):
        nc = self.nc
        probe = nc.sync.nop(nofuse=True)
        wait_clock.add_sem_waits(
            probe.ins, ScopedClock({None: tick_clock.global_clock})
        )
        nc.sync.drain()
        nc.all_engine_barrier()
        assert self.sems is not None
        popped = nc._tile_sem_poison_stack.pop()
        assert popped is self._sem_poison
        nc.clear_and_free_semaphores(list(self.sems.allocated().values()))
        _split_sync_waits(nc)

    tile.TileContext._drain_and_barrier = _drain_and_barrier
    tile.TileContext._drain_patch_applied = True


def build_bass():
    """One SPMD Bass program; every core runs it on its own batch shard."""
    _patch_tile_drain()
    nc = bass.Bass("TRN2", target_bir_lowering=False, debug=False,
                   num_devices=NCORES)
    # x: [g, (hp*64+wp*32+ci), b, hh, s, ww] with s=0 the fp8 value and
    #    s=1 the fp8 residual of xpad[b, 2hh+hp-1, 2ww+wp-1, 32g+ci].
    # Row-interleaved hi/lo planes keep the moving AP's last dim stride-1
    # (a walrus DoubleRow requirement) and DMA descriptors large.
    x = nc.dram_tensor("x", [G, 128, BC, HB, 2, HB], _FP8,
                       kind="ExternalInput")
    # w: [(hp,wp,ci), g, ty(hi/lo), dw, i(=dh pair), (hq*64+wq*32+co)] =
    #    fp8 split of WSCALE*kern[2i+hp-hq, 2dw+wp-wq, ci, 32g+co]
    w = nc.dram_tensor("w", [128, G, 2, 2, 2, 128], _FP8,
                       kind="ExternalInput")
    # y: [g, (hq*64+wq*32+co), b, cc, h7, t] =
    #    WSCALE * out[b, 2*(14cc+h7)+hq, 2t+wq, 32g+co]
    y = nc.dram_tensor("y", [G, 128, BC, 2, 14, 28], _BF16,
                       kind="ExternalOutput")

    with tile.TileContext(nc) as tc:
        with (
            tc.tile_pool(name="wpool", bufs=1) as wpool,
            tc.tile_pool(name="xpool", bufs=1) as xpool,
            tc.tile_pool(name="ypool", bufs=1) as ypool,
            tc.tile_pool(name="psum", bufs=6, space=bass.MemorySpace.PSUM) as pp,
        ):
            wt = wpool.tile([128, G, 2, 2, 2, 128], _FP8, tag="wt")
            xts = {}
            for g in range(G):
                xts[g] = xpool.tile([128, BC, HB, 2, HB], _FP8, tag=f"x{g}",
                                    name=f"xt_{g}")
            ygs = {}
            for g in range(G):
                ygs[g] = ypool.tile([128, BC, 2, 14, 28], _BF16,
                                    tag=f"y{g}", name=f"yg_{g}")

            # Input DMA stream, two issue lanes. The head is bound by the
            # serial HWDGE descriptor-generation pipeline (~650ns per DMA),
            # so all weight DMAs ride the gpsimd SWDGE lane, which generates
            # descriptors on the Pool engine in parallel; the SP/HWDGE lane
            # carries only the input tiles. Group 0 is split finely (rows
            # 0:15 cover the whole first PSUM block) so compute starts as
            # early as possible, and per-b so the PE never outruns the
            # issue-limited head of the stream.
            for g in range(G):
                nc.gpsimd.dma_start(wt[:, g], w[:, g])
            nc.sync.dma_start(xts[0][:, 0, 0:15], x[0, :, 0, 0:15])
            nc.sync.dma_start(xts[0][:, 0, 15:HB], x[0, :, 0, 15:HB])
            nc.sync.dma_start(xts[0][:, 1], x[0, :, 1])
            nc.sync.dma_start(xts[0][:, 2], x[0, :, 2])
            nc.sync.dma_start(xts[0][:, 3], x[0, :, 3])
            for g in range(1, G):
                nc.sync.dma_start(xts[g][:], x[g])

            # Compute: per (g, b, cc) one PSUM block [128, 14, 28], six
            # DoubleRow matmuls: (Wh,xh), (Wh,xl), (Wl,xh) x dw in {0,1}.
            # The DoubleRow pair dim carries the two dh shifts via an
            # aliased-stride view (pair stride == one hh row == HB*2 elems).
            def moving(g, b, cc, h0, h1, dw, s):
                base = xts[g][:]
                off = b * (HB * 2 * HB) + (14 * cc + h0) * (2 * HB) \
                    + s * HB + dw
                return AP(base.tensor, base.offset + off, [
                    list(base.ap[0]),      # partition dim
                    [2 * HB, 2],           # dh pair (aliases the hh axis)
                    [2 * HB, h1 - h0],     # h' rows
                    [1, 28],               # t columns (contiguous)
                ])

            ci = 0
            for g in range(G):
                for b in range(BC):
                    for cc in range(2):
                        ps = pp.tile([128, 14, 28], _F32, tag="ps")
                        terms = [(0, 0, 0), (0, 1, 0),
                                 (0, 0, 1), (0, 1, 1),
                                 (1, 0, 0), (1, 1, 0)]
                        for i, (ty, dw, s) in enumerate(terms):
                            nc.tensor.matmul(
                                ps[:, :, :],
                                wt[:, g, ty, dw],
                                moving(g, b, cc, 0, 14, dw, s),
                                start=(i == 0),
                                stop=(i == len(terms) - 1),
                                perf_mode=_DR,
                            )
                        dst = ygs[g][:, b, cc]
                        if ci % 2 == 0:
                            nc.vector.tensor_copy(dst, ps[:, :, :])
                        else:
                            nc.scalar.copy(dst, ps[:, :, :])
                        ci += 1

            # Output DMAs: one per group (the kernel is DMA-stream-bound;
            # big transfers keep the serial DMA engines ahead of the
            # ~650ns-per-DMA SP issue pipeline). yg covers [128, BC, 2,
            # 14, 28] contiguously per partition.
            for g in range(G):
                nc.sync.dma_start(y[g], ygs[g][:])
    return nc


_NC_CACHE = None


def _get_nc():
    global _NC_CACHE
    if _NC_CACHE is None:
        _NC_CACHE = build_bass()
    return _NC_CACHE


def _fp8_split(a):
    """fp32 array -> (hi, lo) fp8 e4m3 value + residual."""
    hi = a.astype(_FP8NP)
    lo = (a - hi.astype(np.float32)).astype(_FP8NP)
    return hi, lo


def _pack_x(inputs):
    """[B,H,W,C] fp32 -> [G, 128(hp,wp,ci), B, 29, 29, 2] fp8 quadrants."""
    xpad = np.zeros((B, H + 2, W + 2, C), np.float32)
    xpad[:, 1:H + 1, 1:W + 1, :] = inputs
    s = xpad.strides
    # xv[b, hh, hp, ww, wp, g, ci] = xpad[b, 2hh+hp, 2ww+wp, 32g+ci]
    xv = np.lib.stride_tricks.as_strided(
        xpad, shape=(B, HB, 2, HB, 2, G, CPG),
        strides=(s[0], 2 * s[1], s[1], 2 * s[2], s[2], CPG * s[3], s[3]))
    xt = np.ascontiguousarray(
        xv.transpose(5, 2, 4, 6, 0, 1, 3).reshape(G, 128, B, HB, HB))
    hi, lo = _fp8_split(xt)
    return np.stack([hi, lo], axis=-2)  # [G, 128, B, HB, 2, HB]


def _pack_w(kern):
    """HWIO [3,3,32,256] -> [128(hp,wp,ci), g, ty, dw, i, 128(hq,wq,co)]."""
    wd = np.zeros((128, G, 2, 2, 128), np.float32)
    for dh in range(2):
        for dw in range(2):
            for hp in range(2):
                for hq in range(2):
                    kh = 2 * dh + hp - hq
                    if not 0 <= kh < KH:
                        continue
                    for wp in range(2):
                        for wq in range(2):
                            kw = 2 * dw + wp - wq
                            if not 0 <= kw < KW:
                                continue
                            for g in range(G):
                                wd[hp * 64 + wp * 32:hp * 64 + wp * 32 + 32,
                                   g, dw, dh,
                                   hq * 64 + wq * 32:hq * 64 + wq * 32 + 32] \
                                    = WSCALE * kern[kh, kw, :,
                                                    g * CPG:(g + 1) * CPG]
    hi, lo = _fp8_split(wd)
    return np.stack([hi, lo], axis=2)  # [128, G, ty, dw, i, 128]


def _make_in_maps(inputs, kern):
    inputs = np.asarray(inputs, np.float32)
    kern = np.asarray(kern, np.float32)
    xp = _pack_x(inputs)
    wd = _pack_w(kern)
    return [
        {
            "x": np.ascontiguousarray(xp[:, :, c * BC:(c + 1) * BC]),
            "w": wd,
        }
        for c in range(NCORES)
    ]


def _unpack_y(ya):
    """[G,128,BC,2,14,28] bf16 -> [BC,H,W,C] fp32 (descaled)."""
    o = np.asarray(ya, np.float32).reshape(G, 2, 2, CPG, BC, 2, 14, 28)
    # out[b, 2*(14cc+h7)+hq, 2t+wq, 32g+co]
    out = o.transpose(4, 5, 6, 1, 7, 2, 0, 3).reshape(BC, H, W, C)
    return out * (1.0 / WSCALE)


def kernel(inputs, kernel, bias):
    nc = _get_nc()
    in_maps = _make_in_maps(inputs, kernel)
    try:
        res = run_bass_kernel_spmd(nc, in_maps, list(range(NCORES)))
    except ModuleNotFoundError:
        # BASS_TRACE set but the axon NTFF hook module is absent in this
        # container; retry with tracing suppressed.
        import os

        os.environ["BASS_NEVER_TRACE"] = "1"
        res = run_bass_kernel_spmd(nc, in_maps, list(range(NCORES)))

    outs = [_unpack_y(res.results[c]["y"]) for c in range(NCORES)]
    out = np.concatenate(outs, axis=0)
    out = out + np.asarray(bias, np.float32)
    return out.astype(np.float32)


# revision 21
# speedup vs baseline: 1.2583x; 1.0108x over previous
"""Grouped Conv2D (G=8, 3x3, SAME) on 8 TRN2 NeuronCores via Bass/Tile.

Sharding: data-parallel over batch (32 images -> 4 per core).

Quadrant (space-to-depth) scheme, uniform for all 8 groups: SBUF
partitions hold (hp, wp, ci) -- the 4 pixels of a stride-2 2x2 input
block for one group -- with zero input duplication. PSUM partitions
pack (hq, wq, co): the 2x2 output block. Taps decompose over moving
shifts (dh, dw) in {0,1}^2: kh = 2*dh + hp - hq, kw = 2*dw + wp - wq,
each (tap, output) pair covered exactly once.

Compute runs in fp8 (e4m3) DoubleRow matmuls at 2x PE rate, with
error compensation: x = xh + xl (fp8 value + fp8 residual), W split
likewise (scaled by 16 for subnormal headroom; host divides the
output by 16). y = Wh*xh + Wh*xl + Wl*xh leaves only O(eps^2)
error (~3e-3 relative, same as a plain bf16 kernel). DoubleRow
contracts a leading pair dim on both operands (out = sum_i
W[:,i].T @ X[:,i]); the pair dim carries the two dh shifts as an
aliased-stride view of the input tile, so per PSUM block the whole
sum is 6 half-rate matmuls (3 terms x 2 dw) instead of 8.

The kernel is DMA-bound (~39us of HBM traffic vs ~31us of PE), so
the schedule holds the serial DMA engine stream: weight DMAs ride
the gpsimd SWDGE lane (the SP/HWDGE lane's ~650ns-per-DMA issue
pipeline binds the head), input tiles stream per-group with group 0
split finely, and output DMAs sit after all input DMAs in SP
program order so their transfer requests queue behind every input's.
"""

import numpy as np
import ml_dtypes

import concourse.bass as bass
import concourse.mybir as mybir
import concourse.tile as tile
from concourse.ap import AP
from concourse.bass_utils import run_bass_kernel_spmd
from concourse.vector_clock import ScopedClock

# Problem constants (hardcoded per harness contract).
B, H, W, C = 32, 56, 56, 256
G = 8
CPG = C // G  # 32
KH = KW = 3
NCORES = 8
BC = B // NCORES  # 4 batches per core
HB = 29  # padded stride-2 tile extent (rows -1..56 -> 29 pairs)
WSCALE = 16.0  # weight pre-scale; host divides the output by it

_F32 = mybir.dt.float32
_BF16 = mybir.dt.bfloat16
_FP8 = mybir.dt.float8e4
_BF16NP = np.dtype(ml_dtypes.bfloat16)
_FP8NP = np.dtype(ml_dtypes.float8_e4m3)
_DR = mybir.MatmulPerfMode.DoubleRow


def _max_waits(inst):
    # This container's walrus rejects instructions carrying several sync
    # waits ("Too many sync wait commands"); matmul lowers through the
    # LDWEIGHTS struct which is strictest, and the SP drain's NO_STRUCT
    # encoding also rejects them, so give those zero embedded waits.
    if isinstance(inst, (mybir.InstMatmult, mybir.InstDrain)):
        return 0
    return 1


def _split_sync_waits(nc):
    """Hoist excess sync waits onto same-engine nops placed just before
    the owning instruction (program order on one sequencer preserves the
    wait semantics)."""
    cnt = 0
    for bb in nc.m.functions[0].blocks:
        insts = list(bb.instructions)
        if not any(
            inst.sync_info is not None
            and len(inst.sync_info.on_wait) > _max_waits(inst)
            for inst in insts
        ):
            continue
        newl = []
        for inst in insts:
            si = inst.sync_info
            waits = list(si.on_wait) if si is not None else []
            maxw = _max_waits(inst)
            if len(waits) > maxw:
                for wv in waits[maxw:]:
                    cnt += 1
                    nop = mybir.InstNoOp(
                        name=f"waitsplit-{cnt}",
                        engine=inst.engine,
                        bass_nofuse=True,
                        sync_info=mybir.SyncInfo(on_wait=[wv], on_update=[]),
                    )
                    nc.register_instruction(nop, overwrite=True)
                    newl.append(nop)
                inst.sync_info = mybir.SyncInfo(
                    on_wait=waits[:maxw], on_update=list(si.on_update)
                )
            newl.append(inst)
        live = bb.instructions
        live.clear()
        for inst in newl:
            bb.add_instruction(inst)


def _patch_tile_drain():
    if getattr(tile.TileContext, "_drain_patch_applied", False):
        return

    def _drain_and_barrier(self, tick_clock, wait_clock):
        nc = self.nc
        probe = nc.sync.nop(nofuse=True)
        wait_clock.add_sem_waits(
            probe.ins, ScopedClock({None: tick_clock.global_clock})
        )
        nc.sync.drain()
        nc.all_engine_barrier()
        assert self.sems is not None
        popped = nc._tile_sem_poison_stack.pop()
        assert popped is self._sem_poison
        nc.clear_and_free_semaphores(list(self.sems.allocated().values()))
        _split_sync_waits(nc)

    tile.TileContext._drain_and_barrier = _drain_and_barrier
    tile.TileContext._drain_patch_applied = True


def build_bass():
    """One SPMD Bass program; every core runs it on its own batch shard."""
    _patch_tile_drain()
    nc = bass.Bass("TRN2", target_bir_lowering=False, debug=False,
                   num_devices=NCORES)
    # x: [g, (hp*64+wp*32+ci), b, hh, s, ww] with s=0 the fp8 value and
    #    s=1 the fp8 residual of xpad[b, 2hh+hp-1, 2ww+wp-1, 32g+ci].
    # Row-interleaved hi/lo planes keep the moving AP's last dim stride-1
    # (a walrus DoubleRow requirement) and DMA descriptors large.
    x = nc.dram_tensor("x", [G, 128, BC, HB, 2, HB], _FP8,
                       kind="ExternalInput")
    # w: [(hp,wp,ci), g, ty(hi/lo), dw, i(=dh pair), (hq*64+wq*32+co)] =
    #    fp8 split of WSCALE*kern[2i+hp-hq, 2dw+wp-wq, ci, 32g+co]
    w = nc.dram_tensor("w", [128, G, 2, 2, 2, 128], _FP8,
                       kind="ExternalInput")
    # y: [g, (hq*64+wq*32+co), b, cc, h7, t] =
    #    WSCALE * out[b, 2*(14cc+h7)+hq, 2t+wq, 32g+co]
    y = nc.dram_tensor("y", [G, 128, BC, 2, 14, 28], _BF16,
                       kind="ExternalOutput")

    with tile.TileContext(nc) as tc:
        with (
            tc.tile_pool(name="wpool", bufs=1) as wpool,
            tc.tile_pool(name="xpool", bufs=1) as xpool,
            tc.tile_pool(name="ypool", bufs=1) as ypool,
            tc.tile_pool(name="psum", bufs=6, space=bass.MemorySpace.PSUM) as pp,
        ):
            wt = wpool.tile([128, G, 2, 2, 2, 128], _FP8, tag="wt")
            xts = {}
            for g in range(G):
                xts[g] = xpool.tile([128, BC, HB, 2, HB], _FP8, tag=f"x{g}",
                                    name=f"xt_{g}")
            ygs = {}
            for g in range(G):
                ygs[g] = ypool.tile([128, BC, 2, 14, 28], _BF16,
                                    tag=f"y{g}", name=f"yg_{g}")

            # Input DMA stream, two issue lanes. The head is bound by the
            # serial HWDGE descriptor-generation pipeline (~650ns per DMA),
            # so all weight DMAs ride the gpsimd SWDGE lane, which generates
            # descriptors on the Pool engine in parallel; the SP/HWDGE lane
            # carries only the input tiles. Group 0 is split finely (rows
            # 0:15 cover the whole first PSUM block) so compute starts as
            # early as possible, and per-b so the PE never outruns the
            # issue-limited head of the stream.
            # Pool lane: weight slice for group g, then the pad-row memsets
            # for group g+1 (hp=0 partitions never get row hh=0 DMA'd, hp=1
            # never row hh=28 -- both are all-zero SAME padding), so each
            # group's memsets complete well before its input DMA lands.
            # Group 0's weights ride the Activation engine and its first
            # input piece DVE: their preambles end ~400ns before SP's, so
            # the first two transfers start that much sooner.
            nc.scalar.dma_start(wt[:, 0], w[:, 0])
            nc.vector.dma_start(xts[0][:, 0, 0:15], x[0, :, 0, 0:15])
            for g in range(1, G):
                nc.gpsimd.dma_start(wt[:, g], w[:, g])
                nc.gpsimd.memset(xts[g][0:64, :, 0], 0)
                nc.gpsimd.memset(xts[g][64:128, :, HB - 1], 0)
            nc.sync.dma_start(xts[0][:, 0, 15:HB], x[0, :, 0, 15:HB])
            nc.sync.dma_start(xts[0][:, 1], x[0, :, 1])
            nc.sync.dma_start(xts[0][:, 2], x[0, :, 2])
            nc.sync.dma_start(xts[0][:, 3], x[0, :, 3])
            for g in range(1, G):
                nc.sync.dma_start(xts[g][0:64, :, 1:HB], x[g, 0:64, :, 1:HB])
                nc.sync.dma_start(xts[g][64:128, :, 0:HB - 1],
                                  x[g, 64:128, :, 0:HB - 1])

            # Compute: per (g, b, cc) one PSUM block [128, 14, 28], six
            # DoubleRow matmuls: (Wh,xh), (Wh,xl), (Wl,xh) x dw in {0,1}.
            # The DoubleRow pair dim carries the two dh shifts via an
            # aliased-stride view (pair stride == one hh row == HB*2 elems).
            def moving(g, b, cc, h0, h1, dw, s):
                base = xts[g][:]
                off = b * (HB * 2 * HB) + (14 * cc + h0) * (2 * HB) \
                    + s * HB + dw
                return AP(base.tensor, base.offset + off, [
                    list(base.ap[0]),      # partition dim
                    [2 * HB, 2],           # dh pair (aliases the hh axis)
                    [2 * HB, h1 - h0],     # h' rows
                    [1, 28],               # t columns (contiguous)
                ])

            ci = 0
            for g in range(G):
                for b in range(BC):
                    for cc in range(2):
                        ps = pp.tile([128, 14, 28], _F32, tag="ps")
                        terms = [(0, 0, 0), (0, 1, 0),
                                 (0, 0, 1), (0, 1, 1),
                                 (1, 0, 0), (1, 1, 0)]
                        for i, (ty, dw, s) in enumerate(terms):
                            nc.tensor.matmul(
                                ps[:, :, :],
                                wt[:, g, ty, dw],
                                moving(g, b, cc, 0, 14, dw, s),
                                start=(i == 0),
                                stop=(i == len(terms) - 1),
                                perf_mode=_DR,
                            )
                        dst = ygs[g][:, b, cc]
                        if ci % 2 == 0:
                            nc.vector.tensor_copy(dst, ps[:, :, :])
                        else:
                            nc.scalar.copy(dst, ps[:, :, :])
                        ci += 1

            # Output DMAs: one per group (the kernel is DMA-stream-bound;
            # big transfers keep the serial DMA engines ahead of the
            # ~650ns-per-DMA SP issue pipeline). yg covers [128, BC, 2,
            # 14, 28] contiguously per partition.
            for g in range(G):
                nc.sync.dma_start(y[g], ygs[g][:])
    return nc


_NC_CACHE = None


def _get_nc():
    global _NC_CACHE
    if _NC_CACHE is None:
        _NC_CACHE = build_bass()
    return _NC_CACHE


def _fp8_split(a):
    """fp32 array -> (hi, lo) fp8 e4m3 value + residual."""
    hi = a.astype(_FP8NP)
    lo = (a - hi.astype(np.float32)).astype(_FP8NP)
    return hi, lo


def _pack_x(inputs):
    """[B,H,W,C] fp32 -> [G, 128(hp,wp,ci), B, 29, 29, 2] fp8 quadrants."""
    xpad = np.zeros((B, H + 2, W + 2, C), np.float32)
    xpad[:, 1:H + 1, 1:W + 1, :] = inputs
    s = xpad.strides
    # xv[b, hh, hp, ww, wp, g, ci] = xpad[b, 2hh+hp, 2ww+wp, 32g+ci]
    xv = np.lib.stride_tricks.as_strided(
        xpad, shape=(B, HB, 2, HB, 2, G, CPG),
        strides=(s[0], 2 * s[1], s[1], 2 * s[2], s[2], CPG * s[3], s[3]))
    xt = np.ascontiguousarray(
        xv.transpose(5, 2, 4, 6, 0, 1, 3).reshape(G, 128, B, HB, HB))
    hi, lo = _fp8_split(xt)
    return np.stack([hi, lo], axis=-2)  # [G, 128, B, HB, 2, HB]


def _pack_w(kern):
    """HWIO [3,3,32,256] -> [128(hp,wp,ci), g, ty, dw, i, 128(hq,wq,co)]."""
    wd = np.zeros((128, G, 2, 2, 128), np.float32)
    for dh in range(2):
        for dw in range(2):
            for hp in range(2):
                for hq in range(2):
                    kh = 2 * dh + hp - hq
                    if not 0 <= kh < KH:
                        continue
                    for wp in range(2):
                        for wq in range(2):
                            kw = 2 * dw + wp - wq
                            if not 0 <= kw < KW:
                                continue
                            for g in range(G):
                                wd[hp * 64 + wp * 32:hp * 64 + wp * 32 + 32,
                                   g, dw, dh,
                                   hq * 64 + wq * 32:hq * 64 + wq * 32 + 32] \
                                    = WSCALE * kern[kh, kw, :,
                                                    g * CPG:(g + 1) * CPG]
    hi, lo = _fp8_split(wd)
    return np.stack([hi, lo], axis=2)  # [128, G, ty, dw, i, 128]


def _make_in_maps(inputs, kern):
    inputs = np.asarray(inputs, np.float32)
    kern = np.asarray(kern, np.float32)
    xp = _pack_x(inputs)
    wd = _pack_w(kern)
    return [
        {
            "x": np.ascontiguousarray(xp[:, :, c * BC:(c + 1) * BC]),
            "w": wd,
        }
        for c in range(NCORES)
    ]


def _unpack_y(ya):
    """[G,128,BC,2,14,28] bf16 -> [BC,H,W,C] fp32 (descaled)."""
    o = np.asarray(ya, np.float32).reshape(G, 2, 2, CPG, BC, 2, 14, 28)
    # out[b, 2*(14cc+h7)+hq, 2t+wq, 32g+co]
    out = o.transpose(4, 5, 6, 1, 7, 2, 0, 3).reshape(BC, H, W, C)
    return out * (1.0 / WSCALE)


def kernel(inputs, kernel, bias):
    nc = _get_nc()
    in_maps = _make_in_maps(inputs, kernel)
    try:
        res = run_bass_kernel_spmd(nc, in_maps, list(range(NCORES)))
    except ModuleNotFoundError:
        # BASS_TRACE set but the axon NTFF hook module is absent in this
        # container; retry with tracing suppressed.
        import os

        os.environ["BASS_NEVER_TRACE"] = "1"
        res = run_bass_kernel_spmd(nc, in_maps, list(range(NCORES)))

    outs = [_unpack_y(res.results[c]["y"]) for c in range(NCORES)]
    out = np.concatenate(outs, axis=0)
    out = out + np.asarray(bias, np.float32)
    return out.astype(np.float32)


# revision 27
# speedup vs baseline: 1.2598x; 1.0012x over previous
"""Grouped Conv2D (G=8, 3x3, SAME) on 8 TRN2 NeuronCores via Bass/Tile.

Sharding: data-parallel over batch (32 images -> 4 per core).

Quadrant (space-to-depth) scheme, uniform for all 8 groups: SBUF
partitions hold (hp, wp, ci) -- the 4 pixels of a stride-2 2x2 input
block for one group -- with zero input duplication. PSUM partitions
pack (hq, wq, co): the 2x2 output block. Taps decompose over moving
shifts (dh, dw) in {0,1}^2: kh = 2*dh + hp - hq, kw = 2*dw + wp - wq,
each (tap, output) pair covered exactly once.

Compute runs in fp8 (e4m3) DoubleRow matmuls at 2x PE rate, with
error compensation: x = xh + xl (fp8 value + fp8 residual), W split
likewise (scaled by 16 for subnormal headroom; host divides the
output by 16). y = Wh*xh + Wh*xl + Wl*xh leaves only O(eps^2)
error (~3e-3 relative, same as a plain bf16 kernel). DoubleRow
contracts a leading pair dim on both operands (out = sum_i
W[:,i].T @ X[:,i]); the pair dim carries the two dh shifts as an
aliased-stride view of the input tile, so per PSUM block the whole
sum is 6 half-rate matmuls (3 terms x 2 dw) instead of 8.

The kernel is DMA-bound (~39us of HBM traffic vs ~31us of PE), so
the schedule holds the serial DMA engine stream: weight DMAs ride
the gpsimd SWDGE lane (the SP/HWDGE lane's ~650ns-per-DMA issue
pipeline binds the head), input tiles stream per-group with group 0
split finely, and output DMAs sit after all input DMAs in SP
program order so their transfer requests queue behind every input's.
"""

import numpy as np
import ml_dtypes

import concourse.bass as bass
import concourse.mybir as mybir
import concourse.tile as tile
from concourse.ap import AP
from concourse.bass_utils import run_bass_kernel_spmd
from concourse.vector_clock import ScopedClock

# Problem constants (hardcoded per harness contract).
B, H, W, C = 32, 56, 56, 256
G = 8
CPG = C // G  # 32
KH = KW = 3
NCORES = 8
BC = B // NCORES  # 4 batches per core
HB = 29  # padded stride-2 tile extent (rows -1..56 -> 29 pairs)
WSCALE = 16.0  # weight pre-scale; host divides the output by it

_F32 = mybir.dt.float32
_BF16 = mybir.dt.bfloat16
_FP8 = mybir.dt.float8e4
_BF16NP = np.dtype(ml_dtypes.bfloat16)
_FP8NP = np.dtype(ml_dtypes.float8_e4m3)
_DR = mybir.MatmulPerfMode.DoubleRow


def _max_waits(inst):
    # This container's walrus rejects instructions carrying several sync
    # waits ("Too many sync wait commands"); matmul lowers through the
    # LDWEIGHTS struct which is strictest, and the SP drain's NO_STRUCT
    # encoding also rejects them, so give those zero embedded waits.
    if isinstance(inst, (mybir.InstMatmult, mybir.InstDrain)):
        return 0
    return 1


def _split_sync_waits(nc):
    """Hoist excess sync waits onto same-engine nops placed just before
    the owning instruction (program order on one sequencer preserves the
    wait semantics)."""
    cnt = 0
    for bb in nc.m.functions[0].blocks:
        insts = list(bb.instructions)
        if not any(
            inst.sync_info is not None
            and len(inst.sync_info.on_wait) > _max_waits(inst)
            for inst in insts
        ):
            continue
        newl = []
        for inst in insts:
            si = inst.sync_info
            waits = list(si.on_wait) if si is not None else []
            maxw = _max_waits(inst)
            if len(waits) > maxw:
                for wv in waits[maxw:]:
                    cnt += 1
                    nop = mybir.InstNoOp(
                        name=f"waitsplit-{cnt}",
                        engine=inst.engine,
                        bass_nofuse=True,
                        sync_info=mybir.SyncInfo(on_wait=[wv], on_update=[]),
                    )
                    nc.register_instruction(nop, overwrite=True)
                    newl.append(nop)
                inst.sync_info = mybir.SyncInfo(
                    on_wait=waits[:maxw], on_update=list(si.on_update)
                )
            newl.append(inst)
        live = bb.instructions
        live.clear()
        for inst in newl:
            bb.add_instruction(inst)


def _patch_tile_drain():
    if getattr(tile.TileContext, "_drain_patch_applied", False):
        return

    def _drain_and_barrier(self, tick_clock, wait_clock):
        nc = self.nc
        probe = nc.sync.nop(nofuse=True)
        wait_clock.add_sem_waits(
            probe.ins, ScopedClock({None: tick_clock.global_clock})
        )
        nc.sync.drain()
        nc.all_engine_barrier()
        assert self.sems is not None
        popped = nc._tile_sem_poison_stack.pop()
        assert popped is self._sem_poison
        nc.clear_and_free_semaphores(list(self.sems.allocated().values()))
        _split_sync_waits(nc)

    tile.TileContext._drain_and_barrier = _drain_and_barrier
    tile.TileContext._drain_patch_applied = True


def build_bass():
    """One SPMD Bass program; every core runs it on its own batch shard."""
    _patch_tile_drain()
    nc = bass.Bass("TRN2", target_bir_lowering=False, debug=False,
                   num_devices=NCORES)
    # x: [g, (hp*64+wp*32+ci), b, hh, s, ww] with s=0 the fp8 value and
    #    s=1 the fp8 residual of xpad[b, 2hh+hp-1, 2ww+wp-1, 32g+ci].
    # Row-interleaved hi/lo planes keep the moving AP's last dim stride-1
    # (a walrus DoubleRow requirement) and DMA descriptors large.
    x = nc.dram_tensor("x", [G, 128, BC, HB, 2, HB], _FP8,
                       kind="ExternalInput")
    # w: [(hp,wp,ci), g, ty(hi/lo), dw, i(=dh pair), (hq*64+wq*32+co)] =
    #    fp8 split of WSCALE*kern[2i+hp-hq, 2dw+wp-wq, ci, 32g+co]
    w = nc.dram_tensor("w", [128, G, 2, 2, 2, 128], _FP8,
                       kind="ExternalInput")
    # y: [g, (hq*64+wq*32+co), b, cc, h7, t] =
    #    WSCALE * out[b, 2*(14cc+h7)+hq, 2t+wq, 32g+co]
    y = nc.dram_tensor("y", [G, 128, BC, 2, 14, 28], _BF16,
                       kind="ExternalOutput")

    with tile.TileContext(nc) as tc:
        with (
            tc.tile_pool(name="wpool", bufs=1) as wpool,
            tc.tile_pool(name="xpool", bufs=1) as xpool,
            tc.tile_pool(name="ypool", bufs=1) as ypool,
            tc.tile_pool(name="psum", bufs=6, space=bass.MemorySpace.PSUM) as pp,
        ):
            wt = wpool.tile([128, G, 2, 2, 2, 128], _FP8, tag="wt")
            xts = {}
            for g in range(G):
                xts[g] = xpool.tile([128, BC, HB, 2, HB], _FP8, tag=f"x{g}",
                                    name=f"xt_{g}")
            ygs = {}
            for g in range(G):
                ygs[g] = ypool.tile([128, BC, 2, 14, 28], _BF16,
                                    tag=f"y{g}", name=f"yg_{g}")

            # Input DMA stream, two issue lanes. The head is bound by the
            # serial HWDGE descriptor-generation pipeline (~650ns per DMA),
            # so all weight DMAs ride the gpsimd SWDGE lane, which generates
            # descriptors on the Pool engine in parallel; the SP/HWDGE lane
            # carries only the input tiles. Group 0 is split finely (rows
            # 0:15 cover the whole first PSUM block) so compute starts as
            # early as possible, and per-b so the PE never outruns the
            # issue-limited head of the stream.
            # Pool lane: weight slice for group g, then the pad-row memsets
            # for group g+1 (hp=0 partitions never get row hh=0 DMA'd, hp=1
            # never row hh=28 -- both are all-zero SAME padding), so each
            # group's memsets complete well before its input DMA lands.
            nc.gpsimd.dma_start(wt[:, 0], w[:, 0])
            for g in range(1, G):
                nc.gpsimd.dma_start(wt[:, g], w[:, g])
                nc.gpsimd.memset(xts[g][0:64, :, 0], 0)
                nc.gpsimd.memset(xts[g][64:128, :, HB - 1], 0)
            nc.sync.dma_start(xts[0][:, 0, 0:15], x[0, :, 0, 0:15])
            nc.sync.dma_start(xts[0][:, 0, 15:HB], x[0, :, 0, 15:HB])
            nc.sync.dma_start(xts[0][:, 1], x[0, :, 1])
            nc.sync.dma_start(xts[0][:, 2], x[0, :, 2])
            nc.sync.dma_start(xts[0][:, 3], x[0, :, 3])
            for g in range(1, G):
                nc.sync.dma_start(xts[g][0:64, :, 1:HB], x[g, 0:64, :, 1:HB])
                nc.sync.dma_start(xts[g][64:128, :, 0:HB - 1],
                                  x[g, 64:128, :, 0:HB - 1])

            # Compute: per (g, b, cc) one PSUM block [128, 14, 28], six
            # DoubleRow matmuls: (Wh,xh), (Wh,xl), (Wl,xh) x dw in {0,1}.
            # The DoubleRow pair dim carries the two dh shifts via an
            # aliased-stride view (pair stride == one hh row == HB*2 elems).
            def moving(g, b, cc, h0, h1, dw, s):
                base = xts[g][:]
                off = b * (HB * 2 * HB) + (14 * cc + h0) * (2 * HB) \
                    + s * HB + dw
                return AP(base.tensor, base.offset + off, [
                    list(base.ap[0]),      # partition dim
                    [2 * HB, 2],           # dh pair (aliases the hh axis)
                    [2 * HB, h1 - h0],     # h' rows
                    [1, 28],               # t columns (contiguous)
                ])

            ci = 0
            for g in range(G):
                for b in range(BC):
                    for cc in range(2):
                        ps = pp.tile([128, 14, 28], _F32, tag="ps")
                        terms = [(0, 0, 0), (0, 1, 0),
                                 (0, 0, 1), (0, 1, 1),
                                 (1, 0, 0), (1, 1, 0)]
                        for i, (ty, dw, s) in enumerate(terms):
                            nc.tensor.matmul(
                                ps[:, :, :],
                                wt[:, g, ty, dw],
                                moving(g, b, cc, 0, 14, dw, s),
                                start=(i == 0),
                                stop=(i == len(terms) - 1),
                                perf_mode=_DR,
                            )
                        dst = ygs[g][:, b, cc]
                        if ci % 2 == 0:
                            nc.vector.tensor_copy(dst, ps[:, :, :])
                        else:
                            nc.scalar.copy(dst, ps[:, :, :])
                        ci += 1

            # Output DMAs: one per group (the kernel is DMA-stream-bound;
            # big transfers keep the serial DMA engines ahead of the
            # ~650ns-per-DMA SP issue pipeline). yg covers [128, BC, 2,
            # 14, 28] contiguously per partition.
            for g in range(G):
                nc.sync.dma_start(y[g], ygs[g][:])
    return nc


_NC_CACHE = None


def _get_nc():
    global _NC_CACHE
    if _NC_CACHE is None:
        _NC_CACHE = build_bass()
    return _NC_CACHE


def _fp8_split(a):
    """fp32 array -> (hi, lo) fp8 e4m3 value + residual."""
    hi = a.astype(_FP8NP)
    lo = (a - hi.astype(np.float32)).astype(_FP8NP)
    return hi, lo


def _pack_x(inputs):
    """[B,H,W,C] fp32 -> [G, 128(hp,wp,ci), B, 29, 29, 2] fp8 quadrants."""
    xpad = np.zeros((B, H + 2, W + 2, C), np.float32)
    xpad[:, 1:H + 1, 1:W + 1, :] = inputs
    s = xpad.strides
    # xv[b, hh, hp, ww, wp, g, ci] = xpad[b, 2hh+hp, 2ww+wp, 32g+ci]
    xv = np.lib.stride_tricks.as_strided(
        xpad, shape=(B, HB, 2, HB, 2, G, CPG),
        strides=(s[0], 2 * s[1], s[1], 2 * s[2], s[2], CPG * s[3], s[3]))
    xt = np.ascontiguousarray(
        xv.transpose(5, 2, 4, 6, 0, 1, 3).reshape(G, 128, B, HB, HB))
    hi, lo = _fp8_split(xt)
    return np.stack([hi, lo], axis=-2)  # [G, 128, B, HB, 2, HB]


def _pack_w(kern):
    """HWIO [3,3,32,256] -> [128(hp,wp,ci), g, ty, dw, i, 128(hq,wq,co)]."""
    wd = np.zeros((128, G, 2, 2, 128), np.float32)
    for dh in range(2):
        for dw in range(2):
            for hp in range(2):
                for hq in range(2):
                    kh = 2 * dh + hp - hq
                    if not 0 <= kh < KH:
                        continue
                    for wp in range(2):
                        for wq in range(2):
                            kw = 2 * dw + wp - wq
                            if not 0 <= kw < KW:
                                continue
                            for g in range(G):
                                wd[hp * 64 + wp * 32:hp * 64 + wp * 32 + 32,
                                   g, dw, dh,
                                   hq * 64 + wq * 32:hq * 64 + wq * 32 + 32] \
                                    = WSCALE * kern[kh, kw, :,
                                                    g * CPG:(g + 1) * CPG]
    hi, lo = _fp8_split(wd)
    return np.stack([hi, lo], axis=2)  # [128, G, ty, dw, i, 128]


def _make_in_maps(inputs, kern):
    inputs = np.asarray(inputs, np.float32)
    kern = np.asarray(kern, np.float32)
    xp = _pack_x(inputs)
    wd = _pack_w(kern)
    return [
        {
            "x": np.ascontiguousarray(xp[:, :, c * BC:(c + 1) * BC]),
            "w": wd,
        }
        for c in range(NCORES)
    ]


def _unpack_y(ya):
    """[G,128,BC,2,14,28] bf16 -> [BC,H,W,C] fp32 (descaled)."""
    o = np.asarray(ya, np.float32).reshape(G, 2, 2, CPG, BC, 2, 14, 28)
    # out[b, 2*(14cc+h7)+hq, 2t+wq, 32g+co]
    out = o.transpose(4, 5, 6, 1, 7, 2, 0, 3).reshape(BC, H, W, C)
    return out * (1.0 / WSCALE)


def kernel(inputs, kernel, bias):
    nc = _get_nc()
    in_maps = _make_in_maps(inputs, kernel)
    try:
        res = run_bass_kernel_spmd(nc, in_maps, list(range(NCORES)))
    except ModuleNotFoundError:
        # BASS_TRACE set but the axon NTFF hook module is absent in this
        # container; retry with tracing suppressed.
        import os

        os.environ["BASS_NEVER_TRACE"] = "1"
        res = run_bass_kernel_spmd(nc, in_maps, list(range(NCORES)))

    outs = [_unpack_y(res.results[c]["y"]) for c in range(NCORES)]
    out = np.concatenate(outs, axis=0)
    out = out + np.asarray(bias, np.float32)
    return out.astype(np.float32)
